# revision 1
# baseline (speedup 1.0000x reference)
"""ContinuousDeepFM Trainium2 kernel (8-core data-parallel over batch).

Math (algebraically collapsed from the reference — the [B,D,D] interaction
tensor is never materialized):
    fo  = x @ W1 + bias
    xw  = x @ W2
    so[b,j] = 0.5 * xw[b,j]^2 * t[b],  t[b] = sum_i x[b,i]^2 - (sum_i x[b,i])^2
    h   = MLP(x @ Wf)   (3 ReLU layers + final linear, weights mlp_w[i].T)
    out = fo + so + h

Sharding: batch 512 -> 64 rows per core; weights replicated. On-chip layout
is feature-major (activations stored transposed as 4 chunks of 128
partitions) so no on-chip transposes are needed; per-feature biases become
per-partition scalars. t depends only on x, so it is computed host-side in
fp64 and shipped pre-broadcast.

Precision: the output is dominated by the second-order term (RMS ~2e5 vs
~23 for fo and ~1 for h), so W2/x/so stay fp32 while the fo/deep weights
and activations run in fp8e4m3 (measured end-to-end rel err ~4e-6) at 1/4
the weight-DMA bytes.

All device inputs are host-pre-swizzled into dense [128, free] layouts so
every DMA is a contiguous 2D copy.
"""

import numpy as np
import ml_dtypes

B = 512
D = 512
NCORES = 8
BL = B // NCORES  # 64 batch rows per core
P = 128
KC = D // P  # 4 partition chunks of the feature dim

F8 = ml_dtypes.float8_e4m3
BF16 = ml_dtypes.bfloat16

_NC_CACHE = {}


def _split_multi_waits(nc, mybir):
    """This container's walrus build supports only ONE sync wait per
    instruction, but Tile's scheduler attaches several (e.g. the exit
    drain). Split extras into preceding single-wait NoOps on the same
    engine — in-order execution preserves the barrier semantics."""
    ctr = 0
    for fn in nc.m.functions:
        for blk in fn.blocks:
            insts = blk.instructions
            if not any(
                i.sync_info is not None
                and i.sync_info.on_wait
                and len(i.sync_info.on_wait) > 1
                for i in insts
            ):
                continue
            out = []
            for inst in insts:
                si = inst.sync_info
                if si is not None and si.on_wait and len(si.on_wait) > 1:
                    waits = list(si.on_wait)
                    for w in waits[:-1]:
                        ctr += 1
                        nop = mybir.InstNoOp(
                            name=f"wsplit-{ctr}-{inst.name}", ins=[], outs=[]
                        )
                        nop.engine = inst.engine
                        nop.sync_info = mybir.SyncInfo(on_wait=[w], on_update=[])
                        out.append(nop)
                    si.on_wait = [waits[-1]]
                out.append(inst)
            blk.instructions = out
    return ctr


def _build_nc():
    import concourse.bass as bass
    import concourse.mybir as mybir
    import concourse.tile as tile

    dt = mybir.dt
    f32 = dt.float32
    f8 = dt.float8e4
    Alu = mybir.AluOpType

    nc = bass.Bass("TRN2", target_bir_lowering=False, debug=False)

    x_d = nc.dram_tensor("x_d", [P, KC * BL], f32, kind="ExternalInput")
    th_d = nc.dram_tensor("th_d", [P, BL], f32, kind="ExternalInput")
    bias_d = nc.dram_tensor("bias_d", [P, 16], f32, kind="ExternalInput")
    wf_d = nc.dram_tensor("wf_d", [P, KC * D], f8, kind="ExternalInput")
    mw_d = nc.dram_tensor("mw_d", [P, 4 * KC * D], f8, kind="ExternalInput")
    w1_d = nc.dram_tensor("w1_d", [P, KC * D], f8, kind="ExternalInput")
    w2_d = nc.dram_tensor("w2_d", [P, KC * D], f32, kind="ExternalInput")
    out_d = nc.dram_tensor("out_d", [P, KC * BL], f32, kind="ExternalOutput")

    with tile.TileContext(nc) as tc:
        with (
            tc.tile_pool(name="w", bufs=1) as wpool,
            tc.tile_pool(name="act", bufs=1) as apool,
            tc.tile_pool(name="ps", bufs=1, space="PSUM") as pspool,
        ):
            # ---- input DMAs, one dense 2D copy each, split across the two
            # HWDGE rings (sync + scalar) so per-DMA completion-receipt gaps
            # overlap; deep-chain weights on ring A, w2/w1 on ring B.
            xt = apool.tile([P, KC * BL], f32, tag="xt")
            nc.sync.dma_start(xt[:], x_d.ap())
            bias_sb = apool.tile([P, 16], f32, tag="bias")
            nc.scalar.dma_start(bias_sb[:], bias_d.ap())
            th = apool.tile([P, BL], f32, tag="th")
            nc.scalar.dma_start(th[:], th_d.ap())
            wf_sb = wpool.tile([P, KC * D], f8, tag="wf")
            nc.sync.dma_start(wf_sb[:], wf_d.ap())
            w2_sb = wpool.tile([P, KC * D], f32, tag="w2")
            nc.scalar.dma_start(w2_sb[:], w2_d.ap())
            mw_sb = wpool.tile([P, 4 * KC * D], f8, tag="mw")
            for i in range(4):
                nc.sync.dma_start(
                    mw_sb[:, i * KC * D : (i + 1) * KC * D],
                    mw_d.ap()[:, i * KC * D : (i + 1) * KC * D],
                )
            w1_sb = wpool.tile([P, KC * D], f8, tag="w1")
            nc.scalar.dma_start(w1_sb[:], w1_d.ap())

            def wsl(t, kc, jc, base=0):
                return t[:, base + kc * D + jc * P : base + kc * D + (jc + 1) * P]

            def xsl(t, kc):
                return t[:, kc * BL : (kc + 1) * BL]

            # fp8 copy of x for the fo/deep matmuls
            x8 = apool.tile([P, KC * BL], f8, tag="x8")
            nc.vector.tensor_copy(x8[:], xt[:])

            # ---- deep chain (fp8): h0 = x @ Wf
            h_ps = [
                pspool.tile([P, BL], f32, tag="mm", bufs=8, name=f"h0p{j}")
                for j in range(KC)
            ]
            for kc in range(KC):
                for jc in range(KC):
                    nc.tensor.matmul(
                        h_ps[jc][:],
                        wsl(wf_sb, kc, jc),
                        xsl(x8, kc),
                        start=(kc == 0),
                        stop=(kc == KC - 1),
                    )
            h = apool.tile([P, KC * BL], f8, tag="h0")
            for jc in range(KC):
                nc.vector.tensor_copy(xsl(h, jc), h_ps[jc][:])

            # hidden layers 0..1
            for i in range(2):
                l_ps = [
                    pspool.tile([P, BL], f32, tag="mm", bufs=8, name=f"l{i}p{j}")
                    for j in range(KC)
                ]
                for kc in range(KC):
                    for jc in range(KC):
                        nc.tensor.matmul(
                            l_ps[jc][:],
                            wsl(mw_sb, kc, jc, base=i * KC * D),
                            xsl(h, kc),
                            start=(kc == 0),
                            stop=(kc == KC - 1),
                        )
                hn = apool.tile([P, KC * BL], f8, tag=f"h{i + 1}")
                for jc in range(KC):
                    nc.vector.tensor_scalar(
                        xsl(hn, jc),
                        l_ps[jc][:],
                        bias_sb[:, 4 + i * KC + jc : 5 + i * KC + jc],
                        0.0,
                        op0=Alu.add,
                        op1=Alu.max,
                    )
                h = hn

            # ---- xw = x @ W2 (fp32) ; xwsq = xw^2 on ScalarE
            xw_ps = [
                pspool.tile([P, BL], f32, tag="mm", bufs=8, name=f"xw{j}")
                for j in range(KC)
            ]
            for kc in range(KC):
                for jc in range(KC):
                    nc.tensor.matmul(
                        xw_ps[jc][:],
                        wsl(w2_sb, kc, jc),
                        xsl(xt, kc),
                        start=(kc == 0),
                        stop=(kc == KC - 1),
                    )
            xwsq = apool.tile([P, KC * BL], f32, tag="xwsq")
            for jc in range(KC):
                nc.scalar.square(xsl(xwsq, jc), xw_ps[jc][:])

            # so2 = xw^2 * (0.5*t) + btot  (btot = bias + mlp_b[3])
            so = apool.tile([P, KC * BL], f32, tag="so")
            for jc in range(KC):
                nc.vector.tensor_mul(xsl(so, jc), xsl(xwsq, jc), th[:])
            so2 = apool.tile([P, KC * BL], f32, tag="so2")
            for jc in range(KC):
                nc.vector.tensor_scalar(
                    xsl(so2, jc),
                    xsl(so, jc),
                    bias_sb[:, jc : jc + 1],
                    None,
                    op0=Alu.add,
                )

            # hidden layer 2
            i = 2
            l_ps = [
                pspool.tile([P, BL], f32, tag="mm", bufs=8, name=f"l2p{j}")
                for j in range(KC)
            ]
            for kc in range(KC):
                for jc in range(KC):
                    nc.tensor.matmul(
                        l_ps[jc][:],
                        wsl(mw_sb, kc, jc, base=i * KC * D),
                        xsl(h, kc),
                        start=(kc == 0),
                        stop=(kc == KC - 1),
                    )
            hn = apool.tile([P, KC * BL], f8, tag="h3")
            for jc in range(KC):
                nc.vector.tensor_scalar(
                    xsl(hn, jc),
                    l_ps[jc][:],
                    bias_sb[:, 4 + i * KC + jc : 5 + i * KC + jc],
                    0.0,
                    op0=Alu.add,
                    op1=Alu.max,
                )
            h = hn

            # ---- final: o = h3 @ mw[3].T + x @ W1 in one psum group
            o_ps = [
                pspool.tile([P, BL], f32, tag="mm", bufs=8, name=f"op{j}")
                for j in range(KC)
            ]
            for kc in range(KC):
                for jc in range(KC):
                    nc.tensor.matmul(
                        o_ps[jc][:],
                        wsl(mw_sb, kc, jc, base=3 * KC * D),
                        xsl(h, kc),
                        start=(kc == 0),
                        stop=False,
                    )
            for kc in range(KC):
                for jc in range(KC):
                    nc.tensor.matmul(
                        o_ps[jc][:],
                        wsl(w1_sb, kc, jc),
                        xsl(x8, kc),
                        start=False,
                        stop=(kc == KC - 1),
                    )
            out_sb = apool.tile([P, KC * BL], f32, tag="out")
            for jc in range(KC):
                nc.vector.tensor_add(xsl(out_sb, jc), o_ps[jc][:], xsl(so2, jc))

            nc.scalar.dma_start(out_d.ap(), out_sb[:])

    _split_multi_waits(nc, mybir)
    return nc


def _get_nc():
    if "nc" not in _NC_CACHE:
        _NC_CACHE["nc"] = _build_nc()
    return _NC_CACHE["nc"]


def _chunk_major(w):
    """[D, D] lhsT-layout weight -> dense [128, KC*D] chunk-major array."""
    return np.ascontiguousarray(
        w.reshape(KC, P, D).transpose(1, 0, 2).reshape(P, KC * D)
    )


def prepare_in_maps(inputs):
    x = np.asarray(inputs["x"], np.float32)
    w1 = np.asarray(inputs["first_order_weights"], np.float32)
    bias = np.asarray(inputs["bias"], np.float32)
    w2 = np.asarray(inputs["second_order_weights"], np.float32)
    wf = np.asarray(inputs["feature_weights"], np.float32)
    mw = np.asarray(inputs["mlp_w"], np.float32)
    mb = np.asarray(inputs["mlp_b"], np.float32)

    # t[b] = sum x^2 - (sum x)^2 (host, fp64), shipped as 0.5*t broadcast
    xd = x.astype(np.float64)
    t = (xd * xd).sum(1) - xd.sum(1) ** 2
    th_full = (0.5 * t).astype(np.float32)

    w2_dev = _chunk_major(w2)
    wf_dev = _chunk_major(wf).astype(F8)
    w1_dev = _chunk_major(w1).astype(F8)
    # mw[i].T is the lhsT; layer-major, then chunk-major within each layer
    mwT = mw.transpose(0, 2, 1)  # [4, D(k), D(m)]
    mw_dev = np.ascontiguousarray(
        mwT.reshape(4, KC, P, D).transpose(2, 0, 1, 3).reshape(P, 4 * KC * D)
    ).astype(F8)
    # bias_sb layout: [btot(4) | mb0(4) | mb1(4) | mb2(4)]
    btot = (bias + mb[3]).astype(np.float32).reshape(KC, P).T  # [128, 4]
    mb3 = mb[:3].astype(np.float32).reshape(3, KC, P).transpose(2, 0, 1).reshape(P, 12)
    bias_dev = np.ascontiguousarray(np.concatenate([btot, mb3], axis=1))

    in_maps = []
    for c in range(NCORES):
        xs = x[c * BL : (c + 1) * BL, :].T  # [512, 64]
        x_dev = np.ascontiguousarray(
            xs.reshape(KC, P, BL).transpose(1, 0, 2).reshape(P, KC * BL)
        )
        th_dev = np.ascontiguousarray(
            np.broadcast_to(th_full[c * BL : (c + 1) * BL], (P, BL))
        )
        in_maps.append(
            {
                "x_d": x_dev,
                "th_d": th_dev,
                "bias_d": bias_dev,
                "wf_d": wf_dev,
                "mw_d": mw_dev,
                "w1_d": w1_dev,
                "w2_d": w2_dev,
            }
        )
    return in_maps


def assemble_output(results):
    out = np.empty((B, D), np.float32)
    for c in range(NCORES):
        od = results[c]["out_d"]  # [128, KC*BL]
        outT = od.reshape(P, KC, BL).transpose(1, 0, 2).reshape(D, BL)
        out[c * BL : (c + 1) * BL, :] = outT.T
    return out


def kernel(**inputs):
    from concourse.bass_utils import run_bass_kernel_spmd

    nc = _get_nc()
    in_maps = prepare_in_maps(inputs)
    res = run_bass_kernel_spmd(nc, in_maps, core_ids=list(range(NCORES)))
    return assemble_output(res.results)



# revision 2
# speedup vs baseline: 1.2927x; 1.2927x over previous
"""ContinuousDeepFM Trainium2 kernel (8-core data-parallel over batch).

Math (algebraically collapsed from the reference — the [B,D,D] interaction
tensor is never materialized):
    fo  = x @ W1 + bias
    xw  = x @ W2
    so[b,j] = 0.5 * xw[b,j]^2 * t[b],  t[b] = sum_i x[b,i]^2 - (sum_i x[b,i])^2
    h   = MLP(x @ Wf)   (3 ReLU layers + final linear, weights mlp_w[i].T)
    out = fo + so + h

Sharding: batch 512 -> 64 rows per core; weights replicated. On-chip layout
is feature-major (activations stored transposed as 4 chunks of 128
partitions) so no on-chip transposes are needed. t depends only on x, so it
is computed host-side in fp64 and shipped pre-broadcast.

Precision: the output is dominated by the second-order term (RMS ~3e5 vs
~23 for fo and ~1 for h). The so-critical path (x, W2) runs in bf16
(measured end-to-end rel err ~2.8e-3 vs the 2e-2 gate); fo/deep weights and
activations run in fp8e4m3. bias+mlp_b[3] is added via a K=1 matmul into
the output accumulation group (fp8, values ~1e-2, irrelevant to accuracy).

Performance notes (from perfetto/NTFF analysis of the fp32-w2 version):
  - The PE HAM clock gate kept the array at 1.2 GHz nearly the whole
    kernel; a burst of dummy warm-up matmuls at kernel start (no DMA deps)
    flips it to 2.4 GHz before the real matmuls arrive.
  - fp32 matmuls ran LOW_HIGH double-pass at ~427 ns each; bf16 is 1-pass.
  - DMA issue costs ~0.65us per dma_start on a HWDGE ring; inputs are
    packed into few, large, contiguous transfers split across the sync +
    scalar HWDGE rings and the gpsimd SWDGE ring, ordered by compute need.
  - scalar.square pulled a 1.3us ACT_TABLE_LOAD into the scalar ring; the
    so-chain runs on Vector as two tensor_tensor mults per chunk instead.
"""

import numpy as np
import ml_dtypes

B = 512
D = 512
NCORES = 8
BL = B // NCORES  # 64 batch rows per core
P = 128
KC = D // P  # 4 partition chunks of the feature dim

F8 = ml_dtypes.float8_e4m3
BF16 = ml_dtypes.bfloat16

_NC_CACHE = {}

N_WARM = 16  # dummy FD=256 matmuls to lift the HAM clock gate


def _split_multi_waits(nc, mybir):
    """This container's walrus build supports only ONE sync wait per
    instruction, but Tile's scheduler attaches several (e.g. the exit
    drain). Split extras into preceding single-wait NoOps on the same
    engine — in-order execution preserves the barrier semantics."""
    ctr = 0
    for fn in nc.m.functions:
        for blk in fn.blocks:
            insts = blk.instructions
            if not any(
                i.sync_info is not None
                and i.sync_info.on_wait
                and len(i.sync_info.on_wait) > 1
                for i in insts
            ):
                continue
            out = []
            for inst in insts:
                si = inst.sync_info
                if si is not None and si.on_wait and len(si.on_wait) > 1:
                    waits = list(si.on_wait)
                    for w in waits[:-1]:
                        ctr += 1
                        nop = mybir.InstNoOp(
                            name=f"wsplit-{ctr}-{inst.name}", ins=[], outs=[]
                        )
                        nop.engine = inst.engine
                        nop.sync_info = mybir.SyncInfo(on_wait=[w], on_update=[])
                        out.append(nop)
                    si.on_wait = [waits[-1]]
                out.append(inst)
            blk.instructions = out
    return ctr


def _build_nc():
    import concourse.bass as bass
    import concourse.mybir as mybir
    import concourse.tile as tile

    dt = mybir.dt
    f32 = dt.float32
    f8 = dt.float8e4
    bf = dt.bfloat16
    Alu = mybir.AluOpType

    nc = bass.Bass("TRN2", target_bir_lowering=False, debug=False)

    # w8 layout (fp8): [ wf | mw0 | mw1 | mw2 | mw3 | w1 ], 2048 cols each,
    # chunk-major within each block (col kc*D + jc*P + m = chunk [kc->jc]).
    x_d = nc.dram_tensor("x_d", [P, KC * BL], bf, kind="ExternalInput")
    w8_d = nc.dram_tensor("w8_d", [P, 6 * KC * D], f8, kind="ExternalInput")
    w2_d = nc.dram_tensor("w2_d", [P, KC * D], bf, kind="ExternalInput")
    # misc (fp32): cols 0:12 = mlp_b[0..2] chunk-major, cols 12:76 = th bcast
    misc_d = nc.dram_tensor("misc_d", [P, 12 + BL], f32, kind="ExternalInput")
    btot_d = nc.dram_tensor("btot_d", [1, D], f8, kind="ExternalInput")
    out_d = nc.dram_tensor("out_d", [P, KC * BL], f32, kind="ExternalOutput")

    CW = KC * D  # 2048, cols per weight block

    with tile.TileContext(nc) as tc:
        with (
            tc.tile_pool(name="w", bufs=1) as wpool,
            tc.tile_pool(name="act", bufs=1) as apool,
            tc.tile_pool(name="ps", bufs=1, space="PSUM") as pspool,
        ):
            # ---- PE warm-up: dummy matmuls on a memset tile, no DMA deps.
            warm8 = apool.tile([P, 256], f8, tag="warm8")
            nc.gpsimd.memset(warm8[:], 0.0)
            ones8 = apool.tile([1, BL], f8, tag="ones8")
            nc.gpsimd.memset(ones8[:], 1.0)
            warm_ps = pspool.tile([P, 256], f32, tag="warmps")
            for i in range(N_WARM):
                nc.tensor.matmul(
                    warm_ps[:], warm8[:, 0:P], warm8[:], start=True, stop=True
                )

            # ---- input DMAs: three rings (sync + scalar HWDGE, gpsimd
            # SWDGE), issue order = compute-need order, balanced by bytes.
            xbf = apool.tile([P, KC * BL], bf, tag="xbf")
            w8_sb = wpool.tile([P, 6 * CW], f8, tag="w8")
            w2_sb = wpool.tile([P, CW], bf, tag="w2")
            misc = apool.tile([P, 12 + BL], f32, tag="misc")
            btot8 = apool.tile([1, D], f8, tag="btot8")

            # sync ring: x, wf, mw0, mw2
            nc.sync.dma_start(xbf[:], x_d.ap())
            nc.sync.dma_start(w8_sb[:, 0:CW], w8_d.ap()[:, 0:CW])
            nc.sync.dma_start(w8_sb[:, CW : 2 * CW], w8_d.ap()[:, CW : 2 * CW])
            nc.sync.dma_start(
                w8_sb[:, 3 * CW : 4 * CW], w8_d.ap()[:, 3 * CW : 4 * CW]
            )
            # scalar ring: misc, mw1, w2, w1
            nc.scalar.dma_start(misc[:], misc_d.ap())
            nc.scalar.dma_start(
                w8_sb[:, 2 * CW : 3 * CW], w8_d.ap()[:, 2 * CW : 3 * CW]
            )
            nc.scalar.dma_start(w2_sb[:], w2_d.ap())
            nc.scalar.dma_start(
                w8_sb[:, 5 * CW : 6 * CW], w8_d.ap()[:, 5 * CW : 6 * CW]
            )
            # gpsimd ring (SWDGE): mw3, btot
            nc.gpsimd.dma_start(
                w8_sb[:, 4 * CW : 5 * CW], w8_d.ap()[:, 4 * CW : 5 * CW]
            )
            nc.gpsimd.dma_start(btot8[:], btot_d.ap())

            def wsl(blk, kc, jc):
                base = blk * CW
                return w8_sb[:, base + kc * D + jc * P : base + kc * D + (jc + 1) * P]

            def xsl(t, kc):
                return t[:, kc * BL : (kc + 1) * BL]

            th = misc[:, 12 : 12 + BL]

            # fp8 copy of x for the fo/deep matmuls
            x8 = apool.tile([P, KC * BL], f8, tag="x8")
            nc.vector.tensor_copy(x8[:], xbf[:])

            # ---- deep chain (fp8): h0 = x @ Wf  (no bias, no relu)
            h_ps = [
                pspool.tile([P, BL], f32, tag="mm", bufs=7, name=f"h0p{j}")
                for j in range(KC)
            ]
            for kc in range(KC):
                for jc in range(KC):
                    nc.tensor.matmul(
                        h_ps[jc][:],
                        wsl(0, kc, jc),
                        xsl(x8, kc),
                        start=(kc == 0),
                        stop=(kc == KC - 1),
                    )
            h = apool.tile([P, KC * BL], f8, tag="h0")
            for jc in range(KC):
                nc.vector.tensor_copy(xsl(h, jc), h_ps[jc][:])

            # hidden layers 0..2: h = relu(h @ mw[i].T + mb[i])
            for i in range(3):
                l_ps = [
                    pspool.tile([P, BL], f32, tag="mm", bufs=7, name=f"l{i}p{j}")
                    for j in range(KC)
                ]
                for kc in range(KC):
                    for jc in range(KC):
                        nc.tensor.matmul(
                            l_ps[jc][:],
                            wsl(1 + i, kc, jc),
                            xsl(h, kc),
                            start=(kc == 0),
                            stop=(kc == KC - 1),
                        )
                hn = apool.tile([P, KC * BL], f8, tag=f"h{i + 1}")
                for jc in range(KC):
                    nc.vector.tensor_scalar(
                        xsl(hn, jc),
                        l_ps[jc][:],
                        misc[:, i * KC + jc : i * KC + jc + 1],
                        0.0,
                        op0=Alu.add,
                        op1=Alu.max,
                    )
                h = hn

            # ---- xw = x @ W2 (bf16); so = (xw*th)*xw  (th = 0.5*t bcast)
            xw_ps = [
                pspool.tile([P, BL], f32, tag="mm", bufs=7, name=f"xw{j}")
                for j in range(KC)
            ]
            for kc in range(KC):
                for jc in range(KC):
                    nc.tensor.matmul(
                        xw_ps[jc][:],
                        w2_sb[:, kc * D + jc * P : kc * D + (jc + 1) * P],
                        xsl(xbf, kc),
                        start=(kc == 0),
                        stop=(kc == KC - 1),
                    )
            tmp = apool.tile([P, KC * BL], f32, tag="tmp")
            so = apool.tile([P, KC * BL], f32, tag="so")
            for jc in range(KC):
                nc.vector.tensor_mul(xsl(tmp, jc), xw_ps[jc][:], th)
                nc.vector.tensor_mul(xsl(so, jc), xw_ps[jc][:], xsl(tmp, jc))

            # ---- final, jc-major so adds/stores pipeline:
            # o[jc] = h3 @ mw[3].T + x @ W1 + btot  (btot via K=1 matmul)
            out_sb = apool.tile([P, KC * BL], f32, tag="out")
            for jc in range(KC):
                o_ps = pspool.tile([P, BL], f32, tag="mm", bufs=7, name=f"op{jc}")
                for kc in range(KC):
                    nc.tensor.matmul(
                        o_ps[:],
                        wsl(4, kc, jc),
                        xsl(h, kc),
                        start=(kc == 0),
                        stop=False,
                    )
                for kc in range(KC):
                    nc.tensor.matmul(
                        o_ps[:],
                        wsl(5, kc, jc),
                        xsl(x8, kc),
                        start=False,
                        stop=False,
                    )
                nc.tensor.matmul(
                    o_ps[:],
                    btot8[0:1, jc * P : (jc + 1) * P],
                    ones8[:],
                    start=False,
                    stop=True,
                )
                nc.vector.tensor_add(xsl(out_sb, jc), o_ps[:], xsl(so, jc))
                if jc == 1:
                    nc.scalar.dma_start(
                        out_d.ap()[:, 0 : 2 * BL], out_sb[:, 0 : 2 * BL]
                    )
                if jc == 3:
                    nc.sync.dma_start(
                        out_d.ap()[:, 2 * BL : 4 * BL], out_sb[:, 2 * BL : 4 * BL]
                    )

    _split_multi_waits(nc, mybir)
    return nc


def _get_nc():
    if "nc" not in _NC_CACHE:
        _NC_CACHE["nc"] = _build_nc()
    return _NC_CACHE["nc"]


def _chunk_major(w):
    """[D, D] lhsT-layout weight -> dense [128, KC*D] chunk-major array."""
    return np.ascontiguousarray(
        w.reshape(KC, P, D).transpose(1, 0, 2).reshape(P, KC * D)
    )


def prepare_in_maps(inputs):
    x = np.asarray(inputs["x"], np.float32)
    w1 = np.asarray(inputs["first_order_weights"], np.float32)
    bias = np.asarray(inputs["bias"], np.float32)
    w2 = np.asarray(inputs["second_order_weights"], np.float32)
    wf = np.asarray(inputs["feature_weights"], np.float32)
    mw = np.asarray(inputs["mlp_w"], np.float32)
    mb = np.asarray(inputs["mlp_b"], np.float32)

    # t[b] = sum x^2 - (sum x)^2 (host, fp64), shipped as 0.5*t broadcast
    xd = x.astype(np.float64)
    t = (xd * xd).sum(1) - xd.sum(1) ** 2
    th_full = (0.5 * t).astype(np.float32)

    # fp8 weight pack: [ wf | mw0.T | mw1.T | mw2.T | mw3.T | w1 ]
    mwT = mw.transpose(0, 2, 1)  # [4, D(k), D(m)]
    w8_dev = np.ascontiguousarray(
        np.concatenate(
            [_chunk_major(wf)]
            + [_chunk_major(mwT[i]) for i in range(4)]
            + [_chunk_major(w1)],
            axis=1,
        )
    ).astype(F8)
    w2_dev = _chunk_major(w2).astype(BF16)

    # misc: cols 0:12 = mb[0..2] chunk-major ([128,4] per layer), 12:76 = th
    mb3 = mb[:3].astype(np.float32).reshape(3, KC, P).transpose(2, 0, 1).reshape(P, 12)
    btot_dev = (bias + mb[3]).astype(F8).reshape(1, D)

    in_maps = []
    for c in range(NCORES):
        xs = x[c * BL : (c + 1) * BL, :].T  # [512, 64]
        x_dev = np.ascontiguousarray(
            xs.reshape(KC, P, BL).transpose(1, 0, 2).reshape(P, KC * BL)
        ).astype(BF16)
        th_dev = np.broadcast_to(th_full[c * BL : (c + 1) * BL], (P, BL))
        misc_dev = np.ascontiguousarray(
            np.concatenate([mb3, th_dev], axis=1, dtype=np.float32)
        )
        in_maps.append(
            {
                "x_d": x_dev,
                "w8_d": w8_dev,
                "w2_d": w2_dev,
                "misc_d": misc_dev,
                "btot_d": btot_dev,
            }
        )
    return in_maps


def assemble_output(results):
    out = np.empty((B, D), np.float32)
    for c in range(NCORES):
        od = results[c]["out_d"]  # [128, KC*BL]
        outT = od.reshape(P, KC, BL).transpose(1, 0, 2).reshape(D, BL)
        out[c * BL : (c + 1) * BL, :] = outT.T
    return out


def kernel(**inputs):
    from concourse.bass_utils import run_bass_kernel_spmd

    nc = _get_nc()
    in_maps = prepare_in_maps(inputs)
    res = run_bass_kernel_spmd(nc, in_maps, core_ids=list(range(NCORES)))
    return assemble_output(res.results)
